# revision 8
# baseline (speedup 1.0000x reference)
"""MatchNet retrieval-KNN kernel for 8 Trainium2 NeuronCores.

Strategy (candidate-sharded, fp8 DoubleRow device pass + exact fp32/64
host re-score). Scores are computed TRANSPOSED vs the obvious layout:
candidates on SBUF partitions, queries on the moving/free dim.

Host precompute (fp32 BLAS):
  - A = W^T W;  x~_q = A x_q;  g_n = -c_n^T A c_n / 2 (mean-centered).
    Ranking identity: -d^2/2 = x~.c + g + const_q.
  - Rotation trick to fold the per-candidate bias g into the matmul:
    take v = least-variance right-singular direction of X~ (rms(x~.v)
    ~0.09 << score sigma ~30), Householder Q swaps v into coordinate
    255. Rotated queries get slot 255 := t (const), rotated candidates
    get slot 255 := g/t. Then dot' = x~.c - (x~.v)(c.v) + g -- the
    dropped rank-1 term is ~40x below the fp8 noise floor.
  - Quantize both sides to fp8 e4m3 with global scales (max -> 240).

Device (per core, 12800 candidate slots = 100 tiles of 128):
  - 200 DoubleRow fp8 matmuls (K=256 packed as 2x128 pairs, one
    instruction per [128 cand x 512 query] PSUM tile).
  - Selection is a pair-max tree, NOT top-k: for each pair of adjacent
    candidate tiles, ACT copies the even PSUM tile to SBUF bf16 and DVE
    tensor-tensor-max folds the odd PSUM tile into it (TT may read only
    one PSUM operand). Output: bucket maxima [128 part x 2 qb x 50
    pair x 512 q] bf16 -- each bucket = 2 known candidates, so no
    device-side index extraction (no Max8/FindIndex8) is needed at all.

Host merge: 51200 buckets/query pooled over 8 cores; top-96 buckets by
noisy value -> 192 candidates; exact fp64 re-score via s = x~.c - cn2/2;
top-32 -> softmax(-dist) -> weighted sum of candidate_y. Rows whose
rank-32/33 gap is within fp32 ambiguity are re-ranked with
reference-style fp32 arithmetic. Verified: 0/32768 true top-32 missed
in simulation; rel err ~4e-3 vs the fp32 reference.

Toolchain note: walrus rejects >1 sync wait per instruction;
_legalize_waits() peels extra waits onto single-wait same-engine NoOps.
"""

import json
import os
import types

import ml_dtypes
import numpy as np

import concourse.bass as bass
import concourse.mybir as mybir
import concourse.tile as tile
from concourse.bass import ds
from concourse.bass_utils import run_bass_kernel_spmd

B, N, D_IN, DIM, NUMK = 1024, 100000, 256, 512, 32
TEMP = 1.0
NCORES = 8
NSHR = N // NCORES          # 12500 real candidates per core
NSH = 12800                 # padded (100 tiles of 128)
NTILE = NSH // 128          # 100
NPAIR = NTILE // 2          # 50
QB = 512                    # queries per matmul block
NQB = B // QB               # 2
K_B = 96                    # buckets re-scored on host (x2 candidates)
T_SLOT = 4.0                # constant planted in query slot 255

F32 = mybir.dt.float32
BF16 = mybir.dt.bfloat16
FP8 = mybir.dt.float8e4
NP8 = mybir.dt.np(FP8)
ACT_COPY = mybir.ActivationFunctionType.Copy
DR = mybir.MatmulPerfMode.DoubleRow


def _legalize_waits(nc):
    """Wrap nc.to_json_bytes so every instruction carries <=1 sync wait."""
    orig = nc.to_json_bytes

    def patched(self):
        m = json.loads(orig())
        ctr = 0
        for fn in m["functions"]:
            for blk in fn["blocks"]:
                out = []
                for inst in blk["instructions"]:
                    si = inst.get("sync_info")
                    waits = (si or {}).get("on_wait") or []
                    if len(waits) > 1:
                        for w in waits[:-1]:
                            ctr += 1
                            out.append({
                                "debug": inst.get("debug", 0),
                                "engine": inst["engine"],
                                "ins": [],
                                "name": f"I-nopw{ctr}",
                                "opcode": "NoOp",
                                "outs": [],
                                "sync_info": {"on_wait": [w],
                                              "on_update": []},
                            })
                        si["on_wait"] = waits[-1:]
                    out.append(inst)
                blk["instructions"] = out
        return json.dumps(m).encode()

    nc.to_json_bytes = types.MethodType(patched, nc)
    return nc


def _build_bass():
    nc = bass.Bass()
    cx_d = nc.dram_tensor("cx", [128, 2, NSH], FP8, kind="ExternalInput")
    xa_d = nc.dram_tensor("xa", [128, 2, B], FP8, kind="ExternalInput")
    m_d = nc.dram_tensor("m", [128, NQB, NPAIR, QB], BF16,
                         kind="ExternalOutput")

    with (
        tile.TileContext(nc) as tc,
        tc.tile_pool(name="const", bufs=1) as cp,
        tc.tile_pool(name="s", bufs=6) as sp,
        tc.tile_pool(name="ps", bufs=2, space="PSUM") as pp,
    ):
        # order the input DMAs so the first matmul's data lands first
        CHW = NSH // 10                       # 1280 cands (10 tiles) per chunk
        xa = cp.tile([128, 2, B], FP8)
        cxc = [cp.tile([128, 2, CHW], FP8, name=f"cx{ci}") for ci in range(10)]
        nc.sync.dma_start(cxc[0], cx_d[:, :, ds(0, CHW)])
        nc.sync.dma_start(xa[:, :, ds(0, QB)], xa_d[:, :, ds(0, QB)])
        nc.sync.dma_start(xa[:, :, ds(QB, QB)], xa_d[:, :, ds(QB, QB)])
        for ci in range(1, 10):
            nc.sync.dma_start(cxc[ci], cx_d[:, :, ds(ci * CHW, CHW)])

        def cx_slice(tile_idx):
            ci, loc = divmod(tile_idx, 10)
            return cxc[ci][:, :, ds(loc * 128, 128)]

        for j in range(NPAIR):
            pe = pp.tile([128, NQB, QB], F32)   # 2 PSUM banks
            po = pp.tile([128, NQB, QB], F32)
            for qb in range(NQB):
                nc.tensor.matmul(
                    pe[:, qb, :], cx_slice(2 * j),
                    xa[:, :, ds(qb * QB, QB)],
                    start=True, stop=True, perf_mode=DR)
            for qb in range(NQB):
                nc.tensor.matmul(
                    po[:, qb, :], cx_slice(2 * j + 1),
                    xa[:, :, ds(qb * QB, QB)],
                    start=True, stop=True, perf_mode=DR)
            se = sp.tile([128, NQB, QB], BF16, name="se")
            nc.scalar.activation(se, pe, ACT_COPY)
            m = sp.tile([128, NQB, QB], BF16, name="m")
            if j in (15, 31, 47):
                # ACT has slack vs DVE: copy both tiles, TT in bf16 2x mode
                so = sp.tile([128, NQB, QB], BF16, name="so")
                nc.scalar.activation(so, po, ACT_COPY)
                nc.vector.tensor_tensor(m, so, se, mybir.AluOpType.max)
            else:
                nc.vector.tensor_tensor(m, po, se, mybir.AluOpType.max)
            if j % 2 == 0:
                nc.gpsimd.dma_start(m_d[:, :, j, :], m)
            else:
                nc.sync.dma_start(m_d[:, :, j, :], m)
    return _legalize_waits(nc)


_NC_CACHE = {}


def kernel(x, candidate_x, candidate_y, W, b, context_size, is_train):
    x = np.asarray(x, dtype=np.float32)
    candidate_x = np.asarray(candidate_x, dtype=np.float32)
    candidate_y = np.asarray(candidate_y, dtype=np.float32)
    W = np.asarray(W, dtype=np.float32)
    b = np.asarray(b, dtype=np.float32)

    A = (W.T @ W).astype(np.float32)               # [256, 256], symmetric
    XT = x @ A                                     # rows x~_q  [1024, 256]
    Z = candidate_x @ A                            # [N, 256]
    cn2 = np.einsum("ij,ij->i", candidate_x, Z)    # c^T A c
    gc = (-0.5 * cn2)
    gc = (gc - gc.mean()).astype(np.float32)       # centered: ranking same

    # least-variance direction of the query cloud; Householder to slot 255
    _, _, Vt = np.linalg.svd(XT, full_matrices=False)
    v = Vt[-1]
    e = np.zeros(D_IN, dtype=np.float32)
    e[-1] = 1.0
    u = v - e
    u /= np.linalg.norm(u)
    xq = XT - 2.0 * np.outer(XT @ u, u)            # XT @ Q^T (rank-1 form)
    cq = candidate_x - 2.0 * np.outer(candidate_x @ u, u)
    xq[:, -1] = T_SLOT
    cq[:, -1] = gc / T_SLOT

    alpha = np.abs(xq).max() / 240.0
    beta = np.abs(cq).max() / 240.0
    xq8 = np.clip(xq / alpha, -240, 240).astype(NP8)
    cq8 = np.clip(cq / beta, -240, 240).astype(NP8)

    # DoubleRow layouts: [ki, half, n] with dim d = half*128 + ki
    xa_in = np.ascontiguousarray(
        xq8.T.reshape(2, 128, B).transpose(1, 0, 2))
    in_maps = []
    for c in range(NCORES):
        sh = np.zeros((NSH, D_IN), dtype=NP8)
        sh[:NSHR] = cq8[c * NSHR:(c + 1) * NSHR]
        sh[NSHR:, -1] = NP8(-240.0)                # pads lose every bucket
        cx_in = np.ascontiguousarray(
            sh.T.reshape(2, 128, NSH).transpose(1, 0, 2))
        in_maps.append({"cx": cx_in, "xa": xa_in})

    if "nc" not in _NC_CACHE:
        _NC_CACHE["nc"] = _build_bass()
    nc = _NC_CACHE["nc"]

    trace = bool(int(os.environ.get("KERNEL_TRACE", "0")))
    res = run_bass_kernel_spmd(nc, in_maps, core_ids=list(range(NCORES)),
                               trace=trace)
    if trace:
        print(f"HW exec time: {res.exec_time_ns} ns")
        print(f"mean exec time: {res.mean_exec_time_ns} ns")
        if res.instructions_and_trace is not None:
            print("trace:", res.instructions_and_trace[1])

    # ---- host merge: top-K_B buckets, exact re-score of 2*K_B cands ----
    # m[p, qb, j, q] -> bucket value for query qb*512+q, bucket (core,p,j)
    bflat = np.concatenate(
        [r["m"].astype(np.float32).transpose(1, 3, 0, 2).reshape(B, -1)
         for r in res.results], axis=1)            # [B, 8*128*50]

    sel = np.argpartition(-bflat, K_B, axis=1)[:, :K_B]
    rows = np.arange(B)[:, None]
    core = sel // (128 * NPAIR)
    rem = sel % (128 * NPAIR)
    p = rem // NPAIR
    j = rem % NPAIR
    cand0 = core * NSHR + (2 * j) * 128 + p
    cand1 = core * NSHR + (2 * j + 1) * 128 + p
    cand = np.concatenate([cand0, cand1], axis=1)  # [B, 2*K_B]
    lim = np.concatenate([(core + 1) * NSHR] * 2, axis=1)
    valid = cand < lim
    cand = np.where(valid, cand, 0)

    C_sel = candidate_x[cand]                      # [B, 2K, 256]
    s_exact = (np.einsum("rd,rkd->rk", XT, C_sel, dtype=np.float64)
               - 0.5 * cn2[cand])
    s_exact = np.where(valid, s_exact, -1e30)

    ordK = np.argsort(-s_exact, axis=1, kind="stable")
    top = ordK[:, :NUMK]
    s_sel = s_exact[rows, top]
    cand_sel = cand[rows, top]

    xe = (x @ W.T + b).astype(np.float32)
    xn2 = np.sum(xe.astype(np.float64) ** 2, axis=1)
    const_q = x.astype(np.float64) @ (W.T @ b).astype(np.float64) \
        + 0.5 * float(b.astype(np.float64) @ b.astype(np.float64))

    d2 = xn2[:, None] - 2.0 * (s_sel + const_q[:, None])
    d = np.sqrt(np.maximum(d2, 0.0)) / TEMP
    neg = -d
    neg -= neg.max(axis=1, keepdims=True)
    w = np.exp(neg)
    w /= w.sum(axis=1, keepdims=True)
    logits = np.sum(w * candidate_y[cand_sel].astype(np.float64), axis=1)

    # Rows whose rank-32/33 gap is within fp32 rounding ambiguity: re-rank
    # with reference-style fp32 arithmetic so the boundary pick matches.
    gap = (s_exact[rows[:, 0], ordK[:, NUMK - 1]]
           - s_exact[rows[:, 0], ordK[:, NUMK]])
    for r in np.where(gap < 0.01)[0]:
        csel = np.unique(cand[r][valid[r]])
        ce_sel = (candidate_x[csel] @ W.T + b).astype(np.float32)
        sq = (np.sum(xe[r] ** 2, dtype=np.float32)
              + np.sum(ce_sel ** 2, axis=1, dtype=np.float32)
              - 2.0 * (ce_sel @ xe[r]))
        d_r = np.sqrt(np.maximum(sq, 0.0)) / TEMP
        o32 = np.argsort(d_r, kind="stable")[:NUMK]
        nb = (-d_r[o32]).astype(np.float64)
        nb -= nb.max()
        wr = np.exp(nb)
        wr /= wr.sum()
        logits[r] = float(wr @ candidate_y[csel[o32]].astype(np.float64))
    return logits.astype(np.float32)


# revision 12
# speedup vs baseline: 1.0440x; 1.0440x over previous
"""MatchNet retrieval-KNN kernel for 8 Trainium2 NeuronCores.

Strategy (candidate-sharded, fp8 DoubleRow device pass + exact fp32/64
host re-score). Scores are computed TRANSPOSED vs the obvious layout:
candidates on SBUF partitions, queries on the moving/free dim.

Host precompute (fp32 BLAS):
  - A = W^T W;  x~_q = A x_q;  g_n = -c_n^T A c_n / 2 (mean-centered).
    Ranking identity: -d^2/2 = x~.c + g + const_q.
  - Rotation trick to fold the per-candidate bias g into the matmul:
    take v = least-variance right-singular direction of X~ (rms(x~.v)
    ~0.09 << score sigma ~30), Householder Q swaps v into coordinate
    255. Rotated queries get slot 255 := t (const), rotated candidates
    get slot 255 := g/t. Then dot' = x~.c - (x~.v)(c.v) + g -- the
    dropped rank-1 term is ~40x below the fp8 noise floor.
  - Quantize both sides to fp8 e4m3 with global scales (max -> 240).

Device (per core, 12800 candidate slots = 100 tiles of 128):
  - 200 DoubleRow fp8 matmuls (K=256 packed as 2x128 pairs, one
    instruction per [128 cand x 512 query] PSUM tile).
  - Selection is a pair-max tree, NOT top-k: for each pair of adjacent
    candidate tiles, ACT copies the even PSUM tile to SBUF bf16 and DVE
    tensor-tensor-max folds the odd PSUM tile into it (TT may read only
    one PSUM operand). Output: bucket maxima [128 part x 2 qb x 50
    pair x 512 q] bf16 -- each bucket = 2 known candidates, so no
    device-side index extraction (no Max8/FindIndex8) is needed at all.

Host merge: 51200 buckets/query pooled over 8 cores; top-96 buckets by
noisy value -> 192 candidates; exact fp64 re-score via s = x~.c - cn2/2;
top-32 -> softmax(-dist) -> weighted sum of candidate_y. Rows whose
rank-32/33 gap is within fp32 ambiguity are re-ranked with
reference-style fp32 arithmetic. Verified: 0/32768 true top-32 missed
in simulation; rel err ~4e-3 vs the fp32 reference.

Toolchain note: walrus rejects >1 sync wait per instruction;
_legalize_waits() peels extra waits onto single-wait same-engine NoOps.
"""

import json
import os
import types

import ml_dtypes
import numpy as np

import concourse.bass as bass
import concourse.mybir as mybir
import concourse.tile as tile
from concourse.bass import ds
from concourse.bass_utils import run_bass_kernel_spmd

B, N, D_IN, DIM, NUMK = 1024, 100000, 256, 512, 32
TEMP = 1.0
NCORES = 8
NSHR = N // NCORES          # 12500 real candidates per core
NSH = 12800                 # padded (100 tiles of 128)
NTILE = NSH // 128          # 100
NPAIR = NTILE // 2          # 50
QB = 512                    # queries per matmul block
NQB = B // QB               # 2
K_B = 96                    # buckets re-scored on host (x2 candidates)
T_SLOT = 4.0                # constant planted in query slot 255

F32 = mybir.dt.float32
BF16 = mybir.dt.bfloat16
FP8 = mybir.dt.float8e4
NP8 = mybir.dt.np(FP8)
ACT_COPY = mybir.ActivationFunctionType.Copy
DR = mybir.MatmulPerfMode.DoubleRow


def _legalize_waits(nc):
    """Wrap nc.to_json_bytes so every instruction carries <=1 sync wait."""
    orig = nc.to_json_bytes

    def patched(self):
        m = json.loads(orig())
        ctr = 0
        for fn in m["functions"]:
            for blk in fn["blocks"]:
                out = []
                for inst in blk["instructions"]:
                    si = inst.get("sync_info")
                    waits = (si or {}).get("on_wait") or []
                    if len(waits) > 1:
                        for w in waits[:-1]:
                            ctr += 1
                            out.append({
                                "debug": inst.get("debug", 0),
                                "engine": inst["engine"],
                                "ins": [],
                                "name": f"I-nopw{ctr}",
                                "opcode": "NoOp",
                                "outs": [],
                                "sync_info": {"on_wait": [w],
                                              "on_update": []},
                            })
                        si["on_wait"] = waits[-1:]
                    out.append(inst)
                blk["instructions"] = out
        return json.dumps(m).encode()

    nc.to_json_bytes = types.MethodType(patched, nc)
    return nc


def _build_bass():
    nc = bass.Bass()
    cx_d = nc.dram_tensor("cx", [128, 2, NSH], FP8, kind="ExternalInput")
    xa_d = nc.dram_tensor("xa", [128, 2, B], FP8, kind="ExternalInput")
    m_d = nc.dram_tensor("m", [128, NQB, NPAIR, QB], BF16,
                         kind="ExternalOutput")

    with (
        tile.TileContext(nc) as tc,
        tc.tile_pool(name="const", bufs=1) as cp,
        tc.tile_pool(name="s", bufs=6) as sp,
        tc.tile_pool(name="ps", bufs=2, space="PSUM") as pp,
    ):
        # order the input DMAs so the first matmul's data lands first
        CHW = NSH // 10                       # 1280 cands (10 tiles) per chunk
        xa = cp.tile([128, 2, B], FP8)
        cxc = [cp.tile([128, 2, CHW], FP8, name=f"cx{ci}") for ci in range(10)]
        nc.sync.dma_start(cxc[0], cx_d[:, :, ds(0, CHW)])
        nc.sync.dma_start(xa[:, :, ds(0, QB)], xa_d[:, :, ds(0, QB)])
        nc.sync.dma_start(xa[:, :, ds(QB, QB)], xa_d[:, :, ds(QB, QB)])
        for ci in range(1, 10):
            nc.sync.dma_start(cxc[ci], cx_d[:, :, ds(ci * CHW, CHW)])

        def cx_slice(tile_idx):
            ci, loc = divmod(tile_idx, 10)
            return cxc[ci][:, :, ds(loc * 128, 128)]

        for j in range(NPAIR):
            pe = pp.tile([128, NQB, QB], F32)   # 2 PSUM banks
            po = pp.tile([128, NQB, QB], F32)
            for qb in range(NQB):
                nc.tensor.matmul(
                    pe[:, qb, :], cx_slice(2 * j),
                    xa[:, :, ds(qb * QB, QB)],
                    start=True, stop=True, perf_mode=DR)
            for qb in range(NQB):
                nc.tensor.matmul(
                    po[:, qb, :], cx_slice(2 * j + 1),
                    xa[:, :, ds(qb * QB, QB)],
                    start=True, stop=True, perf_mode=DR)
            se = sp.tile([128, NQB, QB], BF16, name="se")
            nc.scalar.activation(se, pe, ACT_COPY)
            m = sp.tile([128, NQB, QB], BF16, name="m")
            nc.vector.tensor_tensor(m, po, se, mybir.AluOpType.max)
            if j % 2 == 0:
                nc.gpsimd.dma_start(m_d[:, :, j, :], m)
            else:
                nc.sync.dma_start(m_d[:, :, j, :], m)
    return _legalize_waits(nc)


_NC_CACHE = {}


def kernel(x, candidate_x, candidate_y, W, b, context_size, is_train):
    x = np.asarray(x, dtype=np.float32)
    candidate_x = np.asarray(candidate_x, dtype=np.float32)
    candidate_y = np.asarray(candidate_y, dtype=np.float32)
    W = np.asarray(W, dtype=np.float32)
    b = np.asarray(b, dtype=np.float32)

    A = (W.T @ W).astype(np.float32)               # [256, 256], symmetric
    XT = x @ A                                     # rows x~_q  [1024, 256]
    Z = candidate_x @ A                            # [N, 256]
    cn2 = np.einsum("ij,ij->i", candidate_x, Z)    # c^T A c
    gc = (-0.5 * cn2)
    gc = (gc - gc.mean()).astype(np.float32)       # centered: ranking same

    # least-variance direction of the query cloud; Householder to slot 255
    _, _, Vt = np.linalg.svd(XT, full_matrices=False)
    v = Vt[-1]
    e = np.zeros(D_IN, dtype=np.float32)
    e[-1] = 1.0
    u = v - e
    u /= np.linalg.norm(u)
    xq = XT - 2.0 * np.outer(XT @ u, u)            # XT @ Q^T (rank-1 form)
    cq = candidate_x - 2.0 * np.outer(candidate_x @ u, u)
    xq[:, -1] = T_SLOT
    cq[:, -1] = gc / T_SLOT

    alpha = np.abs(xq).max() / 240.0
    beta = np.abs(cq).max() / 240.0
    xq8 = np.clip(xq / alpha, -240, 240).astype(NP8)
    cq8 = np.clip(cq / beta, -240, 240).astype(NP8)

    # DoubleRow layouts: [ki, half, n] with dim d = half*128 + ki
    xa_in = np.ascontiguousarray(
        xq8.T.reshape(2, 128, B).transpose(1, 0, 2))
    in_maps = []
    for c in range(NCORES):
        sh = np.zeros((NSH, D_IN), dtype=NP8)
        sh[:NSHR] = cq8[c * NSHR:(c + 1) * NSHR]
        sh[NSHR:, -1] = NP8(-240.0)                # pads lose every bucket
        cx_in = np.ascontiguousarray(
            sh.T.reshape(2, 128, NSH).transpose(1, 0, 2))
        in_maps.append({"cx": cx_in, "xa": xa_in})

    if "nc" not in _NC_CACHE:
        _NC_CACHE["nc"] = _build_bass()
    nc = _NC_CACHE["nc"]

    trace = bool(int(os.environ.get("KERNEL_TRACE", "0")))
    res = run_bass_kernel_spmd(nc, in_maps, core_ids=list(range(NCORES)),
                               trace=trace)
    if trace:
        print(f"HW exec time: {res.exec_time_ns} ns")
        print(f"mean exec time: {res.mean_exec_time_ns} ns")
        if res.instructions_and_trace is not None:
            print("trace:", res.instructions_and_trace[1])

    # ---- host merge: top-K_B buckets, exact re-score of 2*K_B cands ----
    # m[p, qb, j, q] -> bucket value for query qb*512+q, bucket (core,p,j)
    bflat = np.concatenate(
        [r["m"].astype(np.float32).transpose(1, 3, 0, 2).reshape(B, -1)
         for r in res.results], axis=1)            # [B, 8*128*50]

    sel = np.argpartition(-bflat, K_B, axis=1)[:, :K_B]
    rows = np.arange(B)[:, None]
    core = sel // (128 * NPAIR)
    rem = sel % (128 * NPAIR)
    p = rem // NPAIR
    j = rem % NPAIR
    cand0 = core * NSHR + (2 * j) * 128 + p
    cand1 = core * NSHR + (2 * j + 1) * 128 + p
    cand = np.concatenate([cand0, cand1], axis=1)  # [B, 2*K_B]
    lim = np.concatenate([(core + 1) * NSHR] * 2, axis=1)
    valid = cand < lim
    cand = np.where(valid, cand, 0)

    C_sel = candidate_x[cand]                      # [B, 2K, 256]
    s_exact = (np.einsum("rd,rkd->rk", XT, C_sel, dtype=np.float64)
               - 0.5 * cn2[cand])
    s_exact = np.where(valid, s_exact, -1e30)

    ordK = np.argsort(-s_exact, axis=1, kind="stable")
    top = ordK[:, :NUMK]
    s_sel = s_exact[rows, top]
    cand_sel = cand[rows, top]

    xe = (x @ W.T + b).astype(np.float32)
    xn2 = np.sum(xe.astype(np.float64) ** 2, axis=1)
    const_q = x.astype(np.float64) @ (W.T @ b).astype(np.float64) \
        + 0.5 * float(b.astype(np.float64) @ b.astype(np.float64))

    d2 = xn2[:, None] - 2.0 * (s_sel + const_q[:, None])
    d = np.sqrt(np.maximum(d2, 0.0)) / TEMP
    neg = -d
    neg -= neg.max(axis=1, keepdims=True)
    w = np.exp(neg)
    w /= w.sum(axis=1, keepdims=True)
    logits = np.sum(w * candidate_y[cand_sel].astype(np.float64), axis=1)

    # Rows whose rank-32/33 gap is within fp32 rounding ambiguity: re-rank
    # with reference-style fp32 arithmetic so the boundary pick matches.
    gap = (s_exact[rows[:, 0], ordK[:, NUMK - 1]]
           - s_exact[rows[:, 0], ordK[:, NUMK]])
    for r in np.where(gap < 0.01)[0]:
        csel = np.unique(cand[r][valid[r]])
        ce_sel = (candidate_x[csel] @ W.T + b).astype(np.float32)
        sq = (np.sum(xe[r] ** 2, dtype=np.float32)
              + np.sum(ce_sel ** 2, axis=1, dtype=np.float32)
              - 2.0 * (ce_sel @ xe[r]))
        d_r = np.sqrt(np.maximum(sq, 0.0)) / TEMP
        o32 = np.argsort(d_r, kind="stable")[:NUMK]
        nb = (-d_r[o32]).astype(np.float64)
        nb -= nb.max()
        wr = np.exp(nb)
        wr /= wr.sum()
        logits[r] = float(wr @ candidate_y[csel[o32]].astype(np.float64))
    return logits.astype(np.float32)


# revision 13
# speedup vs baseline: 1.0470x; 1.0029x over previous
"""MatchNet retrieval-KNN kernel for 8 Trainium2 NeuronCores.

Strategy (candidate-sharded, fp8 DoubleRow device pass + exact fp32/64
host re-score). Scores are computed TRANSPOSED vs the obvious layout:
candidates on SBUF partitions, queries on the moving/free dim.

Host precompute (fp32 BLAS):
  - A = W^T W;  x~_q = A x_q;  g_n = -c_n^T A c_n / 2 (mean-centered).
    Ranking identity: -d^2/2 = x~.c + g + const_q.
  - Rotation trick to fold the per-candidate bias g into the matmul:
    take v = least-variance right-singular direction of X~ (rms(x~.v)
    ~0.09 << score sigma ~30), Householder Q swaps v into coordinate
    255. Rotated queries get slot 255 := t (const), rotated candidates
    get slot 255 := g/t. Then dot' = x~.c - (x~.v)(c.v) + g -- the
    dropped rank-1 term is ~40x below the fp8 noise floor.
  - Quantize both sides to fp8 e4m3 with global scales (max -> 240).

Device (per core, 12800 candidate slots = 100 tiles of 128):
  - 200 DoubleRow fp8 matmuls (K=256 packed as 2x128 pairs, one
    instruction per [128 cand x 512 query] PSUM tile).
  - Selection is a pair-max tree, NOT top-k: for each pair of adjacent
    candidate tiles, ACT copies the even PSUM tile to SBUF bf16 and DVE
    tensor-tensor-max folds the odd PSUM tile into it (TT may read only
    one PSUM operand). Output: bucket maxima [128 part x 2 qb x 50
    pair x 512 q] bf16 -- each bucket = 2 known candidates, so no
    device-side index extraction (no Max8/FindIndex8) is needed at all.

Host merge: 51200 buckets/query pooled over 8 cores; top-96 buckets by
noisy value -> 192 candidates; exact fp64 re-score via s = x~.c - cn2/2;
top-32 -> softmax(-dist) -> weighted sum of candidate_y. Rows whose
rank-32/33 gap is within fp32 ambiguity are re-ranked with
reference-style fp32 arithmetic. Verified: 0/32768 true top-32 missed
in simulation; rel err ~4e-3 vs the fp32 reference.

Toolchain note: walrus rejects >1 sync wait per instruction;
_legalize_waits() peels extra waits onto single-wait same-engine NoOps.
"""

import json
import os
import types

import ml_dtypes
import numpy as np

import concourse.bass as bass
import concourse.mybir as mybir
import concourse.tile as tile
from concourse.bass import ds
from concourse.bass_utils import run_bass_kernel_spmd

B, N, D_IN, DIM, NUMK = 1024, 100000, 256, 512, 32
TEMP = 1.0
NCORES = 8
NSHR = N // NCORES          # 12500 real candidates per core
NSH = 12800                 # padded (100 tiles of 128)
NTILE = NSH // 128          # 100
NPAIR = NTILE // 2          # 50
QB = 512                    # queries per matmul block
NQB = B // QB               # 2
K_B = 96                    # buckets re-scored on host (x2 candidates)
T_SLOT = 4.0                # constant planted in query slot 255

F32 = mybir.dt.float32
BF16 = mybir.dt.bfloat16
FP8 = mybir.dt.float8e4
NP8 = mybir.dt.np(FP8)
ACT_COPY = mybir.ActivationFunctionType.Copy
DR = mybir.MatmulPerfMode.DoubleRow


def _legalize_waits(nc):
    """Wrap nc.to_json_bytes so every instruction carries <=1 sync wait."""
    orig = nc.to_json_bytes

    def patched(self):
        m = json.loads(orig())
        ctr = 0
        for fn in m["functions"]:
            for blk in fn["blocks"]:
                out = []
                for inst in blk["instructions"]:
                    si = inst.get("sync_info")
                    waits = (si or {}).get("on_wait") or []
                    if len(waits) > 1:
                        for w in waits[:-1]:
                            ctr += 1
                            out.append({
                                "debug": inst.get("debug", 0),
                                "engine": inst["engine"],
                                "ins": [],
                                "name": f"I-nopw{ctr}",
                                "opcode": "NoOp",
                                "outs": [],
                                "sync_info": {"on_wait": [w],
                                              "on_update": []},
                            })
                        si["on_wait"] = waits[-1:]
                    out.append(inst)
                blk["instructions"] = out
        return json.dumps(m).encode()

    nc.to_json_bytes = types.MethodType(patched, nc)
    return nc


def _build_bass():
    nc = bass.Bass()
    cx_d = nc.dram_tensor("cx", [128, 2, NSH], FP8, kind="ExternalInput")
    xa_d = nc.dram_tensor("xa", [128, 2, B], FP8, kind="ExternalInput")
    m_d = nc.dram_tensor("m", [128, NQB, NPAIR, QB], BF16,
                         kind="ExternalOutput")

    with (
        tile.TileContext(nc) as tc,
        tc.tile_pool(name="const", bufs=1) as cp,
        tc.tile_pool(name="s", bufs=10) as sp,
        tc.tile_pool(name="ps", bufs=2, space="PSUM") as pp,
    ):
        # order the input DMAs so the first matmul's data lands first
        CHW = NSH // 10                       # 1280 cands (10 tiles) per chunk
        xa = cp.tile([128, 2, B], FP8)
        cxc = [cp.tile([128, 2, CHW], FP8, name=f"cx{ci}") for ci in range(10)]
        nc.sync.dma_start(cxc[0], cx_d[:, :, ds(0, CHW)])
        nc.sync.dma_start(xa[:, :, ds(0, QB)], xa_d[:, :, ds(0, QB)])
        nc.sync.dma_start(xa[:, :, ds(QB, QB)], xa_d[:, :, ds(QB, QB)])
        for ci in range(1, 10):
            nc.sync.dma_start(cxc[ci], cx_d[:, :, ds(ci * CHW, CHW)])

        def cx_slice(tile_idx):
            ci, loc = divmod(tile_idx, 10)
            return cxc[ci][:, :, ds(loc * 128, 128)]

        for j in range(NPAIR):
            pe = pp.tile([128, NQB, QB], F32)   # 2 PSUM banks
            po = pp.tile([128, NQB, QB], F32)
            for qb in range(NQB):
                nc.tensor.matmul(
                    pe[:, qb, :], cx_slice(2 * j),
                    xa[:, :, ds(qb * QB, QB)],
                    start=True, stop=True, perf_mode=DR)
            for qb in range(NQB):
                nc.tensor.matmul(
                    po[:, qb, :], cx_slice(2 * j + 1),
                    xa[:, :, ds(qb * QB, QB)],
                    start=True, stop=True, perf_mode=DR)
            se = sp.tile([128, NQB, QB], BF16, name="se")
            nc.scalar.activation(se, pe, ACT_COPY)
            m = sp.tile([128, NQB, QB], BF16, name="m")
            nc.vector.tensor_tensor(m, po, se, mybir.AluOpType.max)
            if j % 2 == 0:
                nc.gpsimd.dma_start(m_d[:, :, j, :], m)
            else:
                nc.sync.dma_start(m_d[:, :, j, :], m)
    return _legalize_waits(nc)


_NC_CACHE = {}


def kernel(x, candidate_x, candidate_y, W, b, context_size, is_train):
    x = np.asarray(x, dtype=np.float32)
    candidate_x = np.asarray(candidate_x, dtype=np.float32)
    candidate_y = np.asarray(candidate_y, dtype=np.float32)
    W = np.asarray(W, dtype=np.float32)
    b = np.asarray(b, dtype=np.float32)

    A = (W.T @ W).astype(np.float32)               # [256, 256], symmetric
    XT = x @ A                                     # rows x~_q  [1024, 256]
    Z = candidate_x @ A                            # [N, 256]
    cn2 = np.einsum("ij,ij->i", candidate_x, Z)    # c^T A c
    gc = (-0.5 * cn2)
    gc = (gc - gc.mean()).astype(np.float32)       # centered: ranking same

    # least-variance direction of the query cloud; Householder to slot 255
    _, _, Vt = np.linalg.svd(XT, full_matrices=False)
    v = Vt[-1]
    e = np.zeros(D_IN, dtype=np.float32)
    e[-1] = 1.0
    u = v - e
    u /= np.linalg.norm(u)
    xq = XT - 2.0 * np.outer(XT @ u, u)            # XT @ Q^T (rank-1 form)
    cq = candidate_x - 2.0 * np.outer(candidate_x @ u, u)
    xq[:, -1] = T_SLOT
    cq[:, -1] = gc / T_SLOT

    alpha = np.abs(xq).max() / 240.0
    beta = np.abs(cq).max() / 240.0
    xq8 = np.clip(xq / alpha, -240, 240).astype(NP8)
    cq8 = np.clip(cq / beta, -240, 240).astype(NP8)

    # DoubleRow layouts: [ki, half, n] with dim d = half*128 + ki
    xa_in = np.ascontiguousarray(
        xq8.T.reshape(2, 128, B).transpose(1, 0, 2))
    in_maps = []
    for c in range(NCORES):
        sh = np.zeros((NSH, D_IN), dtype=NP8)
        sh[:NSHR] = cq8[c * NSHR:(c + 1) * NSHR]
        sh[NSHR:, -1] = NP8(-240.0)                # pads lose every bucket
        cx_in = np.ascontiguousarray(
            sh.T.reshape(2, 128, NSH).transpose(1, 0, 2))
        in_maps.append({"cx": cx_in, "xa": xa_in})

    if "nc" not in _NC_CACHE:
        _NC_CACHE["nc"] = _build_bass()
    nc = _NC_CACHE["nc"]

    trace = bool(int(os.environ.get("KERNEL_TRACE", "0")))
    res = run_bass_kernel_spmd(nc, in_maps, core_ids=list(range(NCORES)),
                               trace=trace)
    if trace:
        print(f"HW exec time: {res.exec_time_ns} ns")
        print(f"mean exec time: {res.mean_exec_time_ns} ns")
        if res.instructions_and_trace is not None:
            print("trace:", res.instructions_and_trace[1])

    # ---- host merge: top-K_B buckets, exact re-score of 2*K_B cands ----
    # m[p, qb, j, q] -> bucket value for query qb*512+q, bucket (core,p,j)
    bflat = np.concatenate(
        [r["m"].astype(np.float32).transpose(1, 3, 0, 2).reshape(B, -1)
         for r in res.results], axis=1)            # [B, 8*128*50]

    sel = np.argpartition(-bflat, K_B, axis=1)[:, :K_B]
    rows = np.arange(B)[:, None]
    core = sel // (128 * NPAIR)
    rem = sel % (128 * NPAIR)
    p = rem // NPAIR
    j = rem % NPAIR
    cand0 = core * NSHR + (2 * j) * 128 + p
    cand1 = core * NSHR + (2 * j + 1) * 128 + p
    cand = np.concatenate([cand0, cand1], axis=1)  # [B, 2*K_B]
    lim = np.concatenate([(core + 1) * NSHR] * 2, axis=1)
    valid = cand < lim
    cand = np.where(valid, cand, 0)

    C_sel = candidate_x[cand]                      # [B, 2K, 256]
    s_exact = (np.einsum("rd,rkd->rk", XT, C_sel, dtype=np.float64)
               - 0.5 * cn2[cand])
    s_exact = np.where(valid, s_exact, -1e30)

    ordK = np.argsort(-s_exact, axis=1, kind="stable")
    top = ordK[:, :NUMK]
    s_sel = s_exact[rows, top]
    cand_sel = cand[rows, top]

    xe = (x @ W.T + b).astype(np.float32)
    xn2 = np.sum(xe.astype(np.float64) ** 2, axis=1)
    const_q = x.astype(np.float64) @ (W.T @ b).astype(np.float64) \
        + 0.5 * float(b.astype(np.float64) @ b.astype(np.float64))

    d2 = xn2[:, None] - 2.0 * (s_sel + const_q[:, None])
    d = np.sqrt(np.maximum(d2, 0.0)) / TEMP
    neg = -d
    neg -= neg.max(axis=1, keepdims=True)
    w = np.exp(neg)
    w /= w.sum(axis=1, keepdims=True)
    logits = np.sum(w * candidate_y[cand_sel].astype(np.float64), axis=1)

    # Rows whose rank-32/33 gap is within fp32 rounding ambiguity: re-rank
    # with reference-style fp32 arithmetic so the boundary pick matches.
    gap = (s_exact[rows[:, 0], ordK[:, NUMK - 1]]
           - s_exact[rows[:, 0], ordK[:, NUMK]])
    for r in np.where(gap < 0.01)[0]:
        csel = np.unique(cand[r][valid[r]])
        ce_sel = (candidate_x[csel] @ W.T + b).astype(np.float32)
        sq = (np.sum(xe[r] ** 2, dtype=np.float32)
              + np.sum(ce_sel ** 2, axis=1, dtype=np.float32)
              - 2.0 * (ce_sel @ xe[r]))
        d_r = np.sqrt(np.maximum(sq, 0.0)) / TEMP
        o32 = np.argsort(d_r, kind="stable")[:NUMK]
        nb = (-d_r[o32]).astype(np.float64)
        nb -= nb.max()
        wr = np.exp(nb)
        wr /= wr.sum()
        logits[r] = float(wr @ candidate_y[csel[o32]].astype(np.float64))
    return logits.astype(np.float32)


# revision 15
# speedup vs baseline: 1.0506x; 1.0034x over previous
"""MatchNet retrieval-KNN kernel for 8 Trainium2 NeuronCores.

Strategy (candidate-sharded, fp8 DoubleRow device pass + exact fp32/64
host re-score). Scores are computed TRANSPOSED vs the obvious layout:
candidates on SBUF partitions, queries on the moving/free dim.

Host precompute (fp32 BLAS):
  - A = W^T W;  x~_q = A x_q;  g_n = -c_n^T A c_n / 2 (mean-centered).
    Ranking identity: -d^2/2 = x~.c + g + const_q.
  - Rotation trick to fold the per-candidate bias g into the matmul:
    take v = least-variance right-singular direction of X~ (rms(x~.v)
    ~0.09 << score sigma ~30), Householder Q swaps v into coordinate
    255. Rotated queries get slot 255 := t (const), rotated candidates
    get slot 255 := g/t. Then dot' = x~.c - (x~.v)(c.v) + g -- the
    dropped rank-1 term is ~40x below the fp8 noise floor.
  - Quantize both sides to fp8 e4m3 with global scales (max -> 240).

Device (per core, 12800 candidate slots = 100 tiles of 128):
  - 200 DoubleRow fp8 matmuls (K=256 packed as 2x128 pairs, one
    instruction per [128 cand x 512 query] PSUM tile).
  - Selection is a pair-max tree, NOT top-k: for each pair of adjacent
    candidate tiles, ACT copies the even PSUM tile to SBUF bf16 and DVE
    tensor-tensor-max folds the odd PSUM tile into it (TT may read only
    one PSUM operand). Output: bucket maxima [128 part x 2 qb x 50
    pair x 512 q] bf16 -- each bucket = 2 known candidates, so no
    device-side index extraction (no Max8/FindIndex8) is needed at all.

Host merge: 51200 buckets/query pooled over 8 cores; top-96 buckets by
noisy value -> 192 candidates; exact fp64 re-score via s = x~.c - cn2/2;
top-32 -> softmax(-dist) -> weighted sum of candidate_y. Rows whose
rank-32/33 gap is within fp32 ambiguity are re-ranked with
reference-style fp32 arithmetic. Verified: 0/32768 true top-32 missed
in simulation; rel err ~4e-3 vs the fp32 reference.

Toolchain note: walrus rejects >1 sync wait per instruction;
_legalize_waits() peels extra waits onto single-wait same-engine NoOps.
"""

import json
import os
import types

import ml_dtypes
import numpy as np

import concourse.bass as bass
import concourse.mybir as mybir
import concourse.tile as tile
from concourse.bass import ds
from concourse.bass_utils import run_bass_kernel_spmd

B, N, D_IN, DIM, NUMK = 1024, 100000, 256, 512, 32
TEMP = 1.0
NCORES = 8
NSHR = N // NCORES          # 12500 real candidates per core
NSH = 12800                 # padded (100 tiles of 128)
NTILE = NSH // 128          # 100
NPAIR = NTILE // 2          # 50
QB = 512                    # queries per matmul block
NQB = B // QB               # 2
K_B = 96                    # buckets re-scored on host (x2 candidates)
T_SLOT = 4.0                # constant planted in query slot 255

F32 = mybir.dt.float32
BF16 = mybir.dt.bfloat16
FP8 = mybir.dt.float8e4
NP8 = mybir.dt.np(FP8)
ACT_COPY = mybir.ActivationFunctionType.Copy
DR = mybir.MatmulPerfMode.DoubleRow


def _legalize_waits(nc):
    """Wrap nc.to_json_bytes so every instruction carries <=1 sync wait."""
    orig = nc.to_json_bytes

    def patched(self):
        m = json.loads(orig())
        ctr = 0
        for fn in m["functions"]:
            for blk in fn["blocks"]:
                out = []
                for inst in blk["instructions"]:
                    si = inst.get("sync_info")
                    waits = (si or {}).get("on_wait") or []
                    if len(waits) > 1:
                        for w in waits[:-1]:
                            ctr += 1
                            out.append({
                                "debug": inst.get("debug", 0),
                                "engine": inst["engine"],
                                "ins": [],
                                "name": f"I-nopw{ctr}",
                                "opcode": "NoOp",
                                "outs": [],
                                "sync_info": {"on_wait": [w],
                                              "on_update": []},
                            })
                        si["on_wait"] = waits[-1:]
                    out.append(inst)
                blk["instructions"] = out
        return json.dumps(m).encode()

    nc.to_json_bytes = types.MethodType(patched, nc)
    return nc


def _build_bass():
    nc = bass.Bass()
    cx_d = nc.dram_tensor("cx", [128, 2, NSH], FP8, kind="ExternalInput")
    xa_d = nc.dram_tensor("xa", [128, 2, B], FP8, kind="ExternalInput")
    m_d = nc.dram_tensor("m", [128, NQB, NPAIR, QB], BF16,
                         kind="ExternalOutput")

    with (
        tile.TileContext(nc) as tc,
        tc.tile_pool(name="const", bufs=1) as cp,
        tc.tile_pool(name="s", bufs=10) as sp,
        tc.tile_pool(name="ps", bufs=2, space="PSUM") as pp,
    ):
        # order the input DMAs so the first matmul's data lands first
        CHW = NSH // 10                       # 1280 cands (10 tiles) per chunk
        xa = cp.tile([128, 2, B], FP8)
        cxc = [cp.tile([128, 2, CHW], FP8, name=f"cx{ci}") for ci in range(10)]
        nc.sync.dma_start(cxc[0], cx_d[:, :, ds(0, CHW)])
        nc.sync.dma_start(xa[:, :, ds(0, QB)], xa_d[:, :, ds(0, QB)])
        nc.sync.dma_start(xa[:, :, ds(QB, QB)], xa_d[:, :, ds(QB, QB)])
        for ci in range(1, 10):
            nc.sync.dma_start(cxc[ci], cx_d[:, :, ds(ci * CHW, CHW)])

        def cx_slice(tile_idx):
            ci, loc = divmod(tile_idx, 10)
            return cxc[ci][:, :, ds(loc * 128, 128)]

        for j in range(NPAIR):
            pe = pp.tile([128, NQB, QB], F32)   # 2 PSUM banks
            po = pp.tile([128, NQB, QB], F32)
            for qb in range(NQB):
                nc.tensor.matmul(
                    pe[:, qb, :], cx_slice(2 * j),
                    xa[:, :, ds(qb * QB, QB)],
                    start=True, stop=True, perf_mode=DR)
            for qb in range(NQB):
                nc.tensor.matmul(
                    po[:, qb, :], cx_slice(2 * j + 1),
                    xa[:, :, ds(qb * QB, QB)],
                    start=True, stop=True, perf_mode=DR)
            se = sp.tile([128, NQB, QB], BF16, name="se")
            nc.scalar.activation(se, pe, ACT_COPY)
            m = sp.tile([128, NQB, QB], BF16, name="m")
            nc.vector.tensor_tensor(m, po, se, mybir.AluOpType.max)
            if j % 2 == 0:
                nc.gpsimd.dma_start(m_d[:, :, j, :], m)
            else:
                nc.sync.dma_start(m_d[:, :, j, :], m)
    return _legalize_waits(nc)


def _install_ntff_hook_if_missing():
    """The agent image's antenv lacks axon_hooks; register the ctypes NTFF
    profiling hook so run_bass_kernel_spmd(trace=True) works under axon."""
    import sys
    if "antenv.axon_hooks" in sys.modules:
        return
    try:
        mod = types.ModuleType("antenv.axon_hooks")
        state = {}
        mod.set_axon_ntff_profile_hook = lambda h: state.update(h=h)
        mod.get_axon_ntff_profile_hook = lambda: state.get("h")
        sys.modules["antenv.axon_hooks"] = mod
        import antenv
        antenv.axon_hooks = mod
        from trn_agent_boot.trn_boot import _ntff_profile_via_ctypes
        mod.set_axon_ntff_profile_hook(
            _ntff_profile_via_ctypes("/opt/axon/libaxon_pjrt.so"))
    except Exception:
        pass


_NC_CACHE = {}


def kernel(x, candidate_x, candidate_y, W, b, context_size, is_train):
    x = np.asarray(x, dtype=np.float32)
    candidate_x = np.asarray(candidate_x, dtype=np.float32)
    candidate_y = np.asarray(candidate_y, dtype=np.float32)
    W = np.asarray(W, dtype=np.float32)
    b = np.asarray(b, dtype=np.float32)

    A = (W.T @ W).astype(np.float32)               # [256, 256], symmetric
    XT = x @ A                                     # rows x~_q  [1024, 256]
    Z = candidate_x @ A                            # [N, 256]
    cn2 = np.einsum("ij,ij->i", candidate_x, Z)    # c^T A c
    gc = (-0.5 * cn2)
    gc = (gc - gc.mean()).astype(np.float32)       # centered: ranking same

    # least-variance direction of the query cloud; Householder to slot 255
    _, _, Vt = np.linalg.svd(XT, full_matrices=False)
    v = Vt[-1]
    e = np.zeros(D_IN, dtype=np.float32)
    e[-1] = 1.0
    u = v - e
    u /= np.linalg.norm(u)
    xq = XT - 2.0 * np.outer(XT @ u, u)            # XT @ Q^T (rank-1 form)
    cq = candidate_x - 2.0 * np.outer(candidate_x @ u, u)
    xq[:, -1] = T_SLOT
    cq[:, -1] = gc / T_SLOT

    alpha = np.abs(xq).max() / 240.0
    beta = np.abs(cq).max() / 240.0
    xq8 = np.clip(xq / alpha, -240, 240).astype(NP8)
    cq8 = np.clip(cq / beta, -240, 240).astype(NP8)

    # DoubleRow layouts: [ki, half, n] with dim d = half*128 + ki
    xa_in = np.ascontiguousarray(
        xq8.T.reshape(2, 128, B).transpose(1, 0, 2))
    in_maps = []
    for c in range(NCORES):
        sh = np.zeros((NSH, D_IN), dtype=NP8)
        sh[:NSHR] = cq8[c * NSHR:(c + 1) * NSHR]
        sh[NSHR:, -1] = NP8(-240.0)                # pads lose every bucket
        cx_in = np.ascontiguousarray(
            sh.T.reshape(2, 128, NSH).transpose(1, 0, 2))
        in_maps.append({"cx": cx_in, "xa": xa_in})

    if "nc" not in _NC_CACHE:
        _NC_CACHE["nc"] = _build_bass()
    nc = _NC_CACHE["nc"]

    trace = bool(int(os.environ.get("KERNEL_TRACE", "0")))
    if trace:
        _install_ntff_hook_if_missing()
    try:
        res = run_bass_kernel_spmd(nc, in_maps, core_ids=list(range(NCORES)),
                                   trace=trace)
    except ModuleNotFoundError:
        trace = False
        res = run_bass_kernel_spmd(nc, in_maps, core_ids=list(range(NCORES)),
                                   trace=False)
    if trace:
        print(f"HW exec time: {res.exec_time_ns} ns")
        print(f"mean exec time: {res.mean_exec_time_ns} ns")
        if res.instructions_and_trace is not None:
            print("trace:", res.instructions_and_trace[1])

    # ---- host merge: top-K_B buckets, exact re-score of 2*K_B cands ----
    # m[p, qb, j, q] -> bucket value for query qb*512+q, bucket (core,p,j)
    bflat = np.concatenate(
        [r["m"].astype(np.float32).transpose(1, 3, 0, 2).reshape(B, -1)
         for r in res.results], axis=1)            # [B, 8*128*50]

    sel = np.argpartition(-bflat, K_B, axis=1)[:, :K_B]
    rows = np.arange(B)[:, None]
    core = sel // (128 * NPAIR)
    rem = sel % (128 * NPAIR)
    p = rem // NPAIR
    j = rem % NPAIR
    cand0 = core * NSHR + (2 * j) * 128 + p
    cand1 = core * NSHR + (2 * j + 1) * 128 + p
    cand = np.concatenate([cand0, cand1], axis=1)  # [B, 2*K_B]
    lim = np.concatenate([(core + 1) * NSHR] * 2, axis=1)
    valid = cand < lim
    cand = np.where(valid, cand, 0)

    C_sel = candidate_x[cand]                      # [B, 2K, 256]
    s_exact = (np.einsum("rd,rkd->rk", XT, C_sel, dtype=np.float64)
               - 0.5 * cn2[cand])
    s_exact = np.where(valid, s_exact, -1e30)

    ordK = np.argsort(-s_exact, axis=1, kind="stable")
    top = ordK[:, :NUMK]
    s_sel = s_exact[rows, top]
    cand_sel = cand[rows, top]

    xe = (x @ W.T + b).astype(np.float32)
    xn2 = np.sum(xe.astype(np.float64) ** 2, axis=1)
    const_q = x.astype(np.float64) @ (W.T @ b).astype(np.float64) \
        + 0.5 * float(b.astype(np.float64) @ b.astype(np.float64))

    d2 = xn2[:, None] - 2.0 * (s_sel + const_q[:, None])
    d = np.sqrt(np.maximum(d2, 0.0)) / TEMP
    neg = -d
    neg -= neg.max(axis=1, keepdims=True)
    w = np.exp(neg)
    w /= w.sum(axis=1, keepdims=True)
    logits = np.sum(w * candidate_y[cand_sel].astype(np.float64), axis=1)

    # Rows whose rank-32/33 gap is within fp32 rounding ambiguity: re-rank
    # with reference-style fp32 arithmetic so the boundary pick matches.
    gap = (s_exact[rows[:, 0], ordK[:, NUMK - 1]]
           - s_exact[rows[:, 0], ordK[:, NUMK]])
    for r in np.where(gap < 0.01)[0]:
        csel = np.unique(cand[r][valid[r]])
        ce_sel = (candidate_x[csel] @ W.T + b).astype(np.float32)
        sq = (np.sum(xe[r] ** 2, dtype=np.float32)
              + np.sum(ce_sel ** 2, axis=1, dtype=np.float32)
              - 2.0 * (ce_sel @ xe[r]))
        d_r = np.sqrt(np.maximum(sq, 0.0)) / TEMP
        o32 = np.argsort(d_r, kind="stable")[:NUMK]
        nb = (-d_r[o32]).astype(np.float64)
        nb -= nb.max()
        wr = np.exp(nb)
        wr /= wr.sum()
        logits[r] = float(wr @ candidate_y[csel[o32]].astype(np.float64))
    return logits.astype(np.float32)
